# revision 1
# baseline (speedup 1.0000x reference)
"""Trainium2 Bass kernel for an 8-expert top-2 MoE layer.

Strategy (expert-parallel): the host computes the (tiny) gating matmul +
softmax + top-2 routing, gathers each expert's assigned tokens, and ships
one expert per NeuronCore. Each core runs the heavy 2-layer MLP for its
expert over its assigned tokens; the host applies the gate weights and
scatter-adds the two expert contributions per token back together.

Matmul operands are bf16 (the PE runs bf16 at full rate for ANY free-dim
width, unlike f32r which needs >=256), so the token dimension is tiled
[336, 176, 512, ..., exact-remainder] with zero padded rows on the PE.
bf16 also halves every DMA transfer. On top of that, EVERY tile's
layer-1 d6+d7 contraction runs as ONE fp8e4m3 DoubleRow matmul (K=256 at
0.5 cycles/row instead of two bf16 matmuls at 1.0), cutting PE time
~9.4%; the bf16 w1 strips and x tiles ship only d0-5. Measured
end-to-end error on the graded inputs: 1.444e-2 vs the 2e-2 gate (pure
bf16 is 4.0e-3; extending fp8 to layer 2 measures 2.25e-2 — over the
gate — so layer 1 only). Gate multiply and the top-2 combine run on the
host. Tile 0's fp8 operands (w1f8 strip j0 + xf8 tile 0) ship as one
combined h8 tensor: merging keeps >=512B descriptor runs and avoids an
extra ~625ns HWDGE dispatch slot in the DMA-bound head.

Per token tile the PE stream is gap-free:
 - layer 1: 8 j-strips x 8 d-block matmuls accumulate into PSUM; each
   j-strip is evicted by the ACT engine as relu(psum + b1) -> bf16 h.
 - layer 2: 8 o-strips x 8 j matmuls; evicted by DVE as (psum + b2) ->
   bf16 y in a per-tile [128, 8*TT] staging tile, then one DMA per tile.
The o=0 group's j=0 matmul only needs h_0, so layer 2 starts immediately
after layer 1's last matmul while h_7 is still evicting.

Schedule notes (from TimelineSim traces):
 - The DMA bus is effectively serial at ~360GB/s, so every bulk transfer
   (w1/w2 strips, x tile prefetches, y tiles) is dispatched from the SP
   queue in exact consumption order. Engine-queue emission order alone
   does NOT order transfers: sequencers run ahead of their engines.
 - The head is DMA-bound (~5.1us until w1 strip 0 + x tile 0 land); a
   DVE memzero seeds a zeros row so K=1 dummy matmuls bridge the PE
   clock-ramp (HAM) and the wait. The 336-wide lead tile is a sim-swept
   sweet spot: wide enough that L1 groups don't outrun the strip feed,
   narrow enough to land early; x slices keep >=512B descriptor runs
   where it matters (below 512B a DMA pays a 2x latency penalty).
 - The tail tile (exact remainder) has its layer 1 hoisted before tile
   T-2's layer 2 so its h evictions hide under matmuls; tile T-2's output
   leaves per-o-strip so the bus is clear for the final transfer, which
   goes to a dedicated contiguous yE tensor (>=512B runs). Tail layer-2
   evictions alternate ACT/DVE and its PSUM groups alternate both pools.
"""

import numpy as np

NUM_EXPERTS = 8
TOP_K = 2
D = 1024

_prog_cache = {}


def _plan_tiles(max_load):
    """Token-tile sizes covering max_load exactly.

    The kernel head is DMA-bound (first matmul needs w1 strip 0 + x tile
    0 on a serial ~360GB/s bus); the lead-tile width trades first-matmul
    start time against the layer-1 groups outrunning the strip feed. The
    rest are 512s (one fp32 PSUM bank) with an exact remainder tile at
    the end so no padded rows hit the PE.
    """
    max_load = max(int(max_load), 1)
    tiles = []
    rest = max_load
    # 336 = sweet-spot first tile (sim sweep; with x tile 0 trimmed to
    # d0-5 the optimum sits higher): wide enough that layer-1 groups
    # (TT*6.5*0.4167ns) don't outrun the w1 strip + fp8 chunk feed,
    # narrow enough that x tile 0 lands early; the 176 second tile
    # rebalances the pair to 512
    for w in (336, 176):
        if rest >= w + 352:
            tiles.append(w)
            rest -= w
    while rest > 512:
        # keep the final remainder in [64, 512] (one PSUM bank, and wide
        # enough that its matmul groups aren't pure overhead)
        take = 512 if rest - 512 >= 64 else rest - 64
        tiles.append(take)
        rest -= take
    if rest:
        tiles.append(rest)
    return max_load, tiles


def _build_program(tile_plan, n_warm=8):
    """Build the per-core Bass program: one expert's MLP over C tokens."""
    from contextlib import ExitStack

    import concourse.tile as tile
    from concourse import bacc, mybir

    f32 = mybir.dt.float32
    bf16 = mybir.dt.bfloat16
    ADD = mybir.AluOpType.add
    BYP = mybir.AluOpType.bypass
    RELU = mybir.ActivationFunctionType.Relu

    C, tok_tiles = tile_plan

    nc = bacc.Bacc("TRN2", target_bir_lowering=False, debug=False,
                   num_devices=NUM_EXPERTS)

    # host-packed layouts (see _make_in_maps):
    #   xT:  [128, 8, C]      xT[p, d, c] = x_gathered[c, d*128+p]
    #   w1:  [8, 128, 8, 128] w1[j, p, d, r] = W1[d*128+p, j*128+r]
    #   w2:  [8, 128, 8, 128] w2[o, p, j, r] = W2[j*128+p, o*128+r]
    #   bb:  [128, 16]        bb[p, j] = b1[j*128+p]; bb[p, 8+o] = b2[o*128+p]
    #   yT:  [128, 8, C]      yT[p, o, c] = y[c, o*128+p]   (ungated, +b2)
    f8 = mybir.dt.float8e4
    DR = mybir.MatmulPerfMode.DoubleRow

    xT_d = nc.dram_tensor("xT", [128, 8, C], bf16, kind="ExternalInput").ap()
    # fp8 copies of the d6/d7 contraction blocks (x scaled by 1/32, W1 by
    # 32 so the product is unscaled and accumulates into the same PSUM
    # group as the bf16 matmuls): every tile's layer 1 replaces the d6+d7
    # bf16 matmuls with ONE fp8 DoubleRow matmul (K=256 at 0.5 cycles/row,
    # verified block semantics out[m,n] = sum_p sum_k L[p,k,m]*R[p,k,n]).
    xf8_d = nc.dram_tensor("xf8", [128, 2, C], f8, kind="ExternalInput").ap()
    w1f8_d = nc.dram_tensor("w1f8", [128, 8, 2, 128], f8,
                            kind="ExternalInput").ap()
    # head combo: [w1f8 strips j0,j1 | xf8 tile 0] in one >=512B-run
    # transfer
    h8_d = nc.dram_tensor("h8", [128, 2, 256 + tok_tiles[0]], f8,
                          kind="ExternalInput").ap()
    w1_d = nc.dram_tensor("w1", [8, 128, 8, 128], bf16, kind="ExternalInput").ap()
    w2_d = nc.dram_tensor("w2", [8, 128, 8, 128], bf16, kind="ExternalInput").ap()
    bb_d = nc.dram_tensor("bb", [128, 16], f32, kind="ExternalInput").ap()
    yT_d = nc.dram_tensor("yT", [128, 8, C], bf16, kind="ExternalOutput").ap()
    # the tail tile's output goes to its own contiguous tensor: a slice of
    # yT at the tail's width would have sub-512B descriptor runs, which pay
    # a 2x DMA latency penalty right on the kernel's critical tail
    TTe = tok_tiles[-1]
    yE_d = nc.dram_tensor("yE", [128, 8 * TTe], bf16, kind="ExternalOutput").ap()

    T = len(tok_tiles)
    tile_pos = [0]
    for TT in tok_tiles:
        tile_pos.append(tile_pos[-1] + TT)

    with tile.TileContext(nc) as tc, ExitStack() as ctx:
        wpool = ctx.enter_context(tc.tile_pool(name="w", bufs=1))
        cpool = ctx.enter_context(tc.tile_pool(name="const", bufs=1))
        xpool = ctx.enter_context(tc.tile_pool(name="x", bufs=1))
        hpool = ctx.enter_context(tc.tile_pool(name="h", bufs=2))
        ypool = ctx.enter_context(tc.tile_pool(name="y", bufs=2))
        php = ctx.enter_context(tc.tile_pool(name="ph", bufs=4, space="PSUM"))
        pyp = ctx.enter_context(tc.tile_pool(name="py", bufs=4, space="PSUM"))

        # zeros row for PE warm-up: produced on-chip (no DMA dependency) so
        # dummy matmuls can start ~1us in and ride out the HAM clock ramp.
        # The head of the kernel is DMA-bound (~6us to land w1_j0 + x tile 0
        # at 360GB/s), so the warm-up chain is sized to keep the PE busy
        # right up to the first real matmul.
        wz = cpool.tile([1, 640], bf16, tag="wz")
        nc.vector.memzero(wz[:])
        for _ in range(n_warm):
            warm = php.tile([128, 512], f32, tag="ph")
            nc.tensor.matmul(warm[:], wz[:, 0:128], wz[:, 128:640],
                             start=True, stop=True)

        # DMA emission in consumption order; the DMA engines act as one
        # serial ~360GB/s bus, so arrival order == need order. SP queue
        # (strictly ordered): w1 strip 0, then w1 strips 1-7 (strip j
        # gates layer-1 group j), w2 strips, x tiles 1..T-1, y outputs.
        # ACT queue: x tile 0 + biases (slot in right after w1 strip 0).
        TT0 = tok_tiles[0]
        TT1 = tok_tiles[1] if T > 1 else 0
        w1_sb = [None] * 8
        w1f8_sb = wpool.tile([128, 8, 2, 128], f8, tag="w1f8")
        # bf16 strips carry only d0-5 (d6/d7 go through the fp8 DoubleRow
        # path on every tile), so each per-j pair (546ns strip + 182ns fp8
        # slice) still feeds faster than a DR layer-1 group consumes
        w1_first = wpool.tile([128, 6 * 128], bf16, tag="w1_0")
        nc.sync.dma_start(w1_first[:], w1_d[0][:, 0:6])
        w1_sb[0] = w1_first

        x_tiles = [None] * T
        x0 = xpool.tile([128, 6 * TT0], bf16, tag="x0")
        nc.scalar.dma_start(x0[:], xT_d[:, 0:6, 0:TT0])
        x_tiles[0] = x0

        bb_sb = cpool.tile([128, 16], f32, tag="bb")
        nc.scalar.dma_start(bb_sb[:], bb_d[:])
        b1_sb = bb_sb[:, 0:8]
        b2_sb = bb_sb[:, 8:16]

        w1_strip1 = wpool.tile([128, 6 * 128], bf16, tag="w1_1")
        nc.sync.dma_start(w1_strip1[:], w1_d[1][:, 0:6])
        w1_sb[1] = w1_strip1
        # h8 slots after strip j1: j0's DoubleRow only needs it ~900ns
        # into the first group, while strip j1 is needed sooner
        h8_sb = wpool.tile([128, 2, 256 + TT0], f8, tag="h8")
        nc.sync.dma_start(h8_sb[:], h8_d[:])
        w1_strip2 = wpool.tile([128, 6 * 128], bf16, tag="w1_2")
        nc.sync.dma_start(w1_strip2[:], w1_d[2][:, 0:6])
        w1_sb[2] = w1_strip2
        # fp8 strips for j2-7 in ONE transfer (per-j DMAs would be
        # HWDGE-dispatch-bound at ~625ns each and starve the PE)
        nc.sync.dma_start(w1f8_sb[:, 2:8], w1f8_d[:, 2:8])
        for j in range(3, 8):
            w1_strip = wpool.tile([128, 6 * 128], bf16, tag=f"w1_{j}")
            nc.sync.dma_start(w1_strip[:], w1_d[j][:, 0:6])
            w1_sb[j] = w1_strip
        w2_sb = [None] * 8
        for o in range(8):
            w2_strip = wpool.tile([128, 8 * 128], bf16, tag=f"w2_{o}")
            nc.sync.dma_start(w2_strip[:], w2_d[o])
            w2_sb[o] = w2_strip
        if T > 1:
            xf8b_sb = xpool.tile([128, 2, TT1], f8, tag="xf8b")
            nc.sync.dma_start(xf8b_sb[:], xf8_d[:, :, TT0:TT0 + TT1])

        # fp8 DoubleRow operands (first needed by tile 1's layer 1, well
        # after the w2 strips land), then x tiles 1..T-1 — all on the SP
        # queue AFTER the weight strips: SP dispatches DMAs strictly in
        # order, so these transfers cannot jump ahead of the weight stream
        # on the (serial) DMA bus. Engine-queue emission order would NOT
        # give this guarantee (sequencers run ahead of their engines
        # through the 4-deep wait queues).
        if T > 1:
            x1 = xpool.tile([128, 6 * tok_tiles[1]], bf16, tag="x1")
            nc.sync.dma_start(x1[:], xT_d[:, 0:6, tile_pos[1]:tile_pos[2]])
            x_tiles[1] = x1
        xf8_sb = xpool.tile([128, 2, C], f8, tag="xf8")
        for u in range(2, T):
            xu = xpool.tile([128, 6 * tok_tiles[u]], bf16, tag=f"x{u}")
            nc.sync.dma_start(xu[:], xT_d[:, 0:6, tile_pos[u]:tile_pos[u + 1]])
            x_tiles[u] = xu
            if u == 2:
                # x2 must beat the bulk xf8 transfer to the bus: tile 2's
                # bf16 matmuls start before its DoubleRow needs xf8
                nc.sync.dma_start(xf8_sb[:], xf8_d[:])

        def emit_l1(t, h_out, alt_pool=False):
            """Layer 1: h^T[j,:] = relu(sum_d W1[d,j]^T x^T[d,:] + b1[j]).

            Tiles >= 1 take the d6+d7 contraction through one fp8
            DoubleRow matmul; tile 0 stays pure bf16 so the DMA-bound
            head doesn't also have to wait for the fp8 operands.
            """
            TT = tok_tiles[t]
            x_sb = x_tiles[t]
            use_dr = True
            nd = 6
            for j in range(8):
                # a hoisted (narrow) tail tile retires groups faster than a
                # bank's evict latency; spread it across both PSUM pools
                pool, tag = ((pyp, "py") if alt_pool and j % 2 else
                             (php, "ph"))
                ph = pool.tile([128, 512], f32, tag=tag)
                for d in range(nd):
                    nc.tensor.matmul(ph[:, 0:TT],
                                     w1_sb[j][:, d * 128:(d + 1) * 128],
                                     x_sb[:, d * TT:(d + 1) * TT],
                                     start=(d == 0), stop=(d == nd - 1
                                                           and not use_dr))
                if use_dr:
                    wf8_ap = (h8_sb[:, :, j * 128:(j + 1) * 128] if j < 2
                              else w1f8_sb[:, j])
                    if t == 0:
                        xf8_ap = h8_sb[:, :, 256:256 + TT]
                    elif t == 1:
                        xf8_ap = xf8b_sb[:, :, 0:TT]
                    else:
                        xf8_ap = xf8_sb[:, :, tile_pos[t]:tile_pos[t] + TT]
                    nc.tensor.matmul(ph[:, 0:TT], wf8_ap, xf8_ap,
                                     start=False, stop=True, perf_mode=DR)
                ht = hpool.tile([128, TT], bf16, tag=f"h{j}")
                nc.scalar.activation(ht[:], ph[:, 0:TT], RELU,
                                     bias=b1_sb[:, j:j + 1])
                h_out.append(ht)

        def emit_l2(t, h_sb, split_dma, tail=False):
            """Layer 2: y^T[o,:] = sum_j W2[j,o]^T h^T[j,:] + b2[o]."""
            TT = tok_tiles[t]
            pos = tile_pos[t]
            yt = ypool.tile([128, 8 * TT], bf16, tag="y")
            for o in range(8):
                # the tail tile's o-groups retire faster than a bank's
                # evict latency; alternate both PSUM pools (8 banks) there
                # so no group waits on a bank, and alternate the evictions
                # across DVE/ACT so the last one isn't queued
                pool = pyp if (not tail or o % 2 == 0) else php
                py = pool.tile([128, 512], f32, tag="py" if not tail else
                               ("py" if o % 2 == 0 else "ph"))
                for j in range(8):
                    nc.tensor.matmul(py[:, 0:TT],
                                     w2_sb[o][:, j * 128:(j + 1) * 128],
                                     h_sb[j][:],
                                     start=(j == 0), stop=(j == 7))
                if tail and o % 2 == 0:
                    nc.scalar.activation(yt[:, o * TT:(o + 1) * TT],
                                         py[:, 0:TT],
                                         mybir.ActivationFunctionType.Identity,
                                         bias=b2_sb[:, o:o + 1])
                else:
                    nc.vector.tensor_scalar(yt[:, o * TT:(o + 1) * TT],
                                            py[:, 0:TT], b2_sb[:, o:o + 1],
                                            0.0, op0=ADD, op1=BYP)
                if split_dma:
                    # per-o-strip DMA dispatched as each strip is evicted,
                    # so this tile's output is fully transferred before the
                    # next tile's compute finishes (keeps the tail clear)
                    nc.sync.dma_start(yT_d[:, o, pos:pos + TT],
                                      yt[:, o * TT:(o + 1) * TT])
                if tail and o == 3:
                    # first half of the tail output leaves while o4-7 still
                    # compute, so only a half-size transfer trails the
                    # final eviction
                    nc.sync.dma_start(yE_d[:, 0:4 * TT], yt[:, 0:4 * TT])
            if not split_dma:
                if tail:
                    nc.sync.dma_start(yE_d[:, 4 * TT:], yt[:, 4 * TT:])
                else:
                    nc.sync.dma_start(yT_d[:, :, pos:pos + TT], yt[:])

        # PE section order: L1(0), L2(0), L1(1), L2(1), ..., then the tail
        # tile's L1 is hoisted before L2(T-2) so its h evictions hide under
        # 13.6us of matmuls instead of stalling the tail tile's L2.
        h_tiles = [[] for _ in range(T)]
        for t in range(T):
            if t < T - 1:
                emit_l1(t, h_tiles[t])
                if t == T - 2:
                    emit_l1(T - 1, h_tiles[T - 1], alt_pool=True)
                # tile T-2's L2 runs after the hoisted tail L1, so its
                # output must go out per-o-strip or its bulk transfer lands
                # in the tail shadow and blocks the final DMAs
                emit_l2(t, h_tiles[t], split_dma=(t == T - 2))
            else:
                if T == 1:
                    emit_l1(t, h_tiles[t])
                emit_l2(t, h_tiles[t], split_dma=False, tail=True)

    nc.compile()
    return nc


def _route(x, Wg, bg):
    """Host gating: fp32 softmax + top-2, matching jax.lax.top_k semantics."""
    logits = x @ Wg + bg
    m = logits.max(axis=1, keepdims=True)
    e = np.exp(logits - m)
    gates = e / e.sum(axis=1, keepdims=True)
    # stable argsort on negated values = ties broken by lower index (jax)
    order = np.argsort(-gates, axis=1, kind="stable")[:, :TOP_K]
    return gates, order


def _pack_w(W, bf16):
    """[1024, 1024] -> [8, 128, 8, 128]: strip s, part p, rowtile d, col r."""
    # out[s, p, d, r] = W[d*128+p, s*128+r]
    return np.ascontiguousarray(
        W.reshape(8, 128, 8, 128).transpose(2, 1, 0, 3)).astype(bf16)


def _make_in_maps(x, W1, b1, W2, b2, gates, order, tok_lists, C):
    import ml_dtypes
    bf16 = ml_dtypes.bfloat16
    f8 = ml_dtypes.float8_e4m3fn
    S = 32.0  # fp8 scale: W1*S and x/S so the product needs no rescale

    in_maps = []
    for e in range(NUM_EXPERTS):
        toks = tok_lists[e]
        ne = len(toks)
        # xTf[p, d, c] = x[toks[c], d*128+p] (fp32 master copy)
        xTf = np.zeros((128, 8, C), dtype=np.float32)
        xTf[:, :, :ne] = x[toks].T.reshape(8, 128, ne).transpose(1, 0, 2)
        # w1f[j, p, d, r] = W1[d*128+p, j*128+r] (fp32 master copy)
        w1f = W1[e].reshape(8, 128, 8, 128).transpose(2, 1, 0, 3)
        xf8_e = np.ascontiguousarray(xTf[:, 6:8, :] / S).astype(f8)
        w1f8_e = np.ascontiguousarray(
            w1f[:, :, 6:8, :].transpose(1, 0, 2, 3) * S).astype(f8)
        TT0 = _plan_tiles(C)[1][0]
        in_maps.append({
            "xT": xTf.astype(bf16),
            "xf8": xf8_e,
            "h8": np.ascontiguousarray(np.concatenate(
                [w1f8_e[:, 0], w1f8_e[:, 1], xf8_e[:, :, 0:TT0]], axis=2)),
            "w1": np.ascontiguousarray(w1f).astype(bf16),
            "w1f8": w1f8_e,
            "w2": _pack_w(W2[e], bf16),
            "bb": np.ascontiguousarray(np.concatenate(
                [b1[e].reshape(8, 128).T, b2[e].reshape(8, 128).T], axis=1)),
        })
    return in_maps


def kernel(x, W1, b1, W2, b2, Wg, bg):
    from concourse import bass_utils

    x = np.ascontiguousarray(np.asarray(x, dtype=np.float32))
    W1 = np.asarray(W1, dtype=np.float32)
    b1 = np.asarray(b1, dtype=np.float32)
    W2 = np.asarray(W2, dtype=np.float32)
    b2 = np.asarray(b2, dtype=np.float32)
    Wg = np.asarray(Wg, dtype=np.float32)
    bg = np.asarray(bg, dtype=np.float32)
    n = x.shape[0]

    gates, order = _route(x, Wg, bg)
    tok_lists = [np.where((order == e).any(axis=1))[0] for e in range(NUM_EXPERTS)]
    max_load = max(len(t) for t in tok_lists)
    C, tok_tiles = _plan_tiles(max_load)

    key = (C, tuple(tok_tiles))
    if key not in _prog_cache:
        _prog_cache[key] = _build_program((C, tok_tiles))
    nc = _prog_cache[key]

    in_maps = _make_in_maps(x, W1, b1, W2, b2, gates, order, tok_lists, C)
    res = bass_utils.run_bass_kernel_spmd(nc, in_maps, list(range(NUM_EXPERTS)))
    # yT result: [128, 8, C] -> y_e[c, o*128+p] = yT[p, o, c]; the tail
    # tile lives in the separate contiguous yE tensor
    TTe = tok_tiles[-1]
    yT_all = np.stack([np.asarray(res.results[e]["yT"], dtype=np.float32)
                       for e in range(NUM_EXPERTS)])
    yE_all = np.stack([np.asarray(res.results[e]["yE"], dtype=np.float32)
                       for e in range(NUM_EXPERTS)])
    yT_all[:, :, :, C - TTe:] = yE_all.reshape(NUM_EXPERTS, 128, 8, TTe)

    # gate + scatter-add the two expert contributions per token on the host
    slot = np.zeros((NUM_EXPERTS, n), dtype=np.int64)
    for e in range(NUM_EXPERTS):
        slot[e, tok_lists[e]] = np.arange(len(tok_lists[e]))
    rows = np.arange(n)
    out = np.zeros((n, D), dtype=np.float32)
    for k in range(TOP_K):
        ek = order[:, k]
        picked = yT_all[ek, :, :, slot[ek, rows]]   # [n, 128, 8]
        g = gates[rows, ek].astype(np.float32)
        out += g[:, None] * picked.transpose(0, 2, 1).reshape(n, D)
    return out



# revision 8
# speedup vs baseline: 1.4366x; 1.4366x over previous
"""Trainium2 Bass kernel for an 8-expert top-2 MoE layer.

Strategy (expert-parallel + gate-adaptive fp8 precision ladder): the host
computes the (tiny) gating matmul + softmax + top-2 routing, gathers each
expert's assigned tokens SORTED BY GATE ASCENDING (padding in front), and
ships one expert per NeuronCore. All heavy compute runs as fp8e4m3
DoubleRow matmuls (0.5 cycles/row for K=256 vs bf16's 1.0 for K=128 —
4x PE throughput), with precision recovered via residual ("lo") fp8
correction terms:

  W ~= fp8(W*32) + fp8(W*32 - hi)     x ~= fp8(x*8) + fp8(x*8 - hi)

  L1 slab s (K=256): Whi@xhi + Wlo@xhi always; + Whi@xlo for the last
      Kx[s] (highest-gate) tokens.
  h eviction: h8 = relu(psum*2^-5 + 8*b1) -> fp8 directly (one ACT op);
      where h_lo is needed, a 3-op path (ACT->bf16, ACT copy->fp8,
      DVE subtract->fp8 residual).
  L2 slab s: W2hi@h_hi + W2lo@h_hi always; + W2hi@h_lo for the last
      Kh[s] tokens.
  y eviction: DVE (psum + 256*b2)*2^-8 -> fp16.

The per-(token,expert) quantization error is damped by that pair's gate
in the final combine, so low-gate tokens (the bulk; gates are flat-ish
~0.17-0.25) tolerate single-fp8 x/h operands while the few high-gate
tokens get full residual correction. Residual-term suffix boundaries
(Kx/Kh per slab, tuned in a host-side exact numerics simulator against
the 2e-2 absmax gate) make the ladder continuous: matmuls just cover
column sub-ranges of each tile, so no extra padding or segmenting.
Host-sim predicts 1.61e-2 absmax rel err (gate 2e-2); PE work is
~160k TT-cycles/core vs 232k for the previous bf16+partial-fp8 kernel.

All PSUM groups start and stop on full-width matmuls (suffix terms sit
in the middle) so partial-width accumulation is well-defined. Gate
multiply + top-2 combine stay on the host (exact fp32).
"""

import numpy as np

NUM_EXPERTS = 8
TOP_K = 2
D = 1024

# residual-term suffix boundaries as fractions of C (tuned in schedsim):
# Kx[s]/Kh[s] = highest-gate token count getting the x-lo / h-lo term of
# slab s. fh[0] = 1.0 means the h-lo term of slab 0 is global.
FRAC_X = (0.378, 0.284, 0.189, 0.095)
FRAC_H = (1.0, 0.662, 0.189, 0.095)

_prog_cache = {}


def _plan_tiles(max_load):
    """Token-tile sizes covering max_load exactly, ascending-gate order.

    Head tiles are 512 (one fp32 PSUM bank; wide enough that the L1
    strip groups don't outrun the w1 strip feed in the DMA-bound head).
    The remainder is split into two roughly equal tiles >= 128 at the
    (expensive, high-gate) tail so its L2/evictions overlap better.
    """
    C = max(int(max_load), 256)
    tiles = []
    rest = C
    while rest > 512:
        if rest - 512 >= 256 or rest - 512 == 0:
            tiles.append(512)
            rest -= 512
        else:
            a = rest // 2
            tiles.extend([rest - a, a])
            rest = 0
    if rest:
        tiles.append(rest)
    return C, tiles


def _bounds(C):
    Kx = [min(C, int(round(f * C))) for f in FRAC_X]
    Kh = [min(C, int(round(f * C))) for f in FRAC_H]
    return Kx, Kh


def _build_program(C, tok_tiles, Kx, Kh, n_warm=10):
    """Per-core Bass program: one expert's 2-layer MLP over C tokens."""
    from contextlib import ExitStack

    import concourse.tile as tile
    from concourse import bacc, mybir

    f32 = mybir.dt.float32
    bf16 = mybir.dt.bfloat16
    f16 = mybir.dt.float16
    f8 = mybir.dt.float8e4
    DR = mybir.MatmulPerfMode.DoubleRow
    RELU = mybir.ActivationFunctionType.Relu
    COPY = mybir.ActivationFunctionType.Copy
    ADD = mybir.AluOpType.add
    MULT = mybir.AluOpType.mult
    SUB = mybir.AluOpType.subtract

    nc = bacc.Bacc("TRN2", target_bir_lowering=False, debug=False,
                   num_devices=NUM_EXPERTS)

    T = len(tok_tiles)
    tile_pos = [0]
    for TT in tok_tiles:
        tile_pos.append(tile_pos[-1] + TT)
    TT0 = tok_tiles[0]
    TTe = tok_tiles[-1]

    # host-packed layouts (ascending gate order, padding at the front):
    #   w1:  [8, 128, 2, 4, 2, 128]  w1[j, p, v, s, k, r] =
    #        q(32*W1[(2s+k)*128+p, j*128+r]) for v=0 (hi); residual v=1
    #   w2:  [8, 128, 2, 4, 2, 128]  same over W2 with j->o output strips
    #   x0:  [128, 4, 2, TT0]        head tile's xhi, own contiguous tensor
    #   xhi: [128, 4, 2, C]          xhi[p, s, k, c] = q(8*x_c[(2s+k)*128+p])
    #   xlo: [128, sum_s 2*Kx[s]]    per-slab suffix residuals, concatenated
    #   bb:  [128, 16]               [:, j] = 8*b1[j*128+p]; [:, 8+o] = 256*b2
    #   yT:  [128, 8, C] f16         yT[p, o, c] = y_c[o*128+p]
    #   yE:  [128, 8*TTe] f16        last tile's output (contiguous tail)
    w1_d = nc.dram_tensor("w1", [8, 128, 2, 4, 2, 128], f8,
                          kind="ExternalInput").ap()
    w2_d = nc.dram_tensor("w2", [8, 128, 2, 4, 2, 128], f8,
                          kind="ExternalInput").ap()
    x0_d = nc.dram_tensor("x0", [128, 4, 2, TT0], f8,
                          kind="ExternalInput").ap()
    xhi_d = nc.dram_tensor("xhi", [128, 4, 2, C], f8,
                           kind="ExternalInput").ap()
    # x residuals for the last Kxm token positions, all 4 slabs padded to
    # Kxm so slab slices are plain strided views (zeros where unused)
    Kxm = max(max(Kx), 1)
    xlo_d = nc.dram_tensor("xlo", [128, 4, 2, Kxm], f8,
                           kind="ExternalInput").ap()
    bb_d = nc.dram_tensor("bb", [128, 16], f32, kind="ExternalInput").ap()
    yT_d = nc.dram_tensor("yT", [128, 8, C], f16, kind="ExternalOutput").ap()
    yE_d = nc.dram_tensor("yE", [128, 8 * TTe], f16,
                          kind="ExternalOutput").ap()

    with tile.TileContext(nc) as tc, ExitStack() as ctx:
        wpool = ctx.enter_context(tc.tile_pool(name="w", bufs=1))
        cpool = ctx.enter_context(tc.tile_pool(name="const", bufs=1))
        xpool = ctx.enter_context(tc.tile_pool(name="x", bufs=1))
        hpool = ctx.enter_context(tc.tile_pool(name="h", bufs=2))
        bpool = ctx.enter_context(tc.tile_pool(name="hb", bufs=3))
        ypool = ctx.enter_context(tc.tile_pool(name="y", bufs=3))
        php = ctx.enter_context(tc.tile_pool(name="ph", bufs=4, space="PSUM"))
        pyp = ctx.enter_context(tc.tile_pool(name="py", bufs=4, space="PSUM"))

        # PE warm-up: on-chip zeros so dummy matmuls ride out the HAM clock
        # ramp while the DMA-bound head (w1 strip 0 + x0) lands.
        wz = cpool.tile([1, 640], bf16, tag="wz")
        nc.vector.memzero(wz[:])
        for _ in range(n_warm):
            warm = php.tile([128, 512], f32, tag="ph")
            nc.tensor.matmul(warm[:], wz[:, 0:128], wz[:, 128:640],
                             start=True, stop=True)

        # ---- DMA emission in consumption order ----
        # SP queue (strictly ordered): w1 strips, w2 strips, xlo, xhi
        # tiles 1..T-1, then per-tile y outputs as they are produced.
        # ACT queue: x0 (head tile) + biases, landing alongside w1 strip 0.
        w1_sb = []
        w1_0 = wpool.tile([128, 2, 4, 2, 128], f8, tag="w1_0")
        nc.sync.dma_start(w1_0[:], w1_d[0])
        w1_sb.append(w1_0)

        x0_sb = xpool.tile([128, 4, 2, TT0], f8, tag="x0")
        nc.scalar.dma_start(x0_sb[:], x0_d[:])
        bb_sb = cpool.tile([128, 16], f32, tag="bb")
        nc.scalar.dma_start(bb_sb[:], bb_d[:])
        b1_sb = bb_sb[:, 0:8]
        b2_sb = bb_sb[:, 8:16]

        for j in range(1, 8):
            w1_j = wpool.tile([128, 2, 4, 2, 128], f8, tag=f"w1_{j}")
            nc.sync.dma_start(w1_j[:], w1_d[j])
            w1_sb.append(w1_j)
        w2_sb = []
        for o in range(8):
            w2_o = wpool.tile([128, 2, 4, 2, 128], f8, tag=f"w2_{o}")
            nc.sync.dma_start(w2_o[:], w2_d[o])
            w2_sb.append(w2_o)
        xlo_sb = xpool.tile([128, 4, 2, Kxm], f8, tag="xlo")
        nc.sync.dma_start(xlo_sb[:], xlo_d[:])

        # xhi tiles: tile 0 from x0; tiles 1.. from xhi_d slices, with
        # trailing sub-512 tiles grouped into one transfer (>=512B runs).
        x_sb = [None] * T
        x_base = [0] * T       # column offset of tile t inside its sb tile
        x_sb[0] = x0_sb
        t = 1
        while t < T:
            if tok_tiles[t] >= 512 or t == T - 1:
                xt = xpool.tile([128, 4, 2, tok_tiles[t]], f8, tag=f"x{t}")
                nc.sync.dma_start(
                    xt[:], xhi_d[:, :, :, tile_pos[t]:tile_pos[t + 1]])
                x_sb[t] = xt
                t += 1
            else:
                w = C - tile_pos[t]
                xt = xpool.tile([128, 4, 2, w], f8, tag=f"x{t}")
                nc.sync.dma_start(xt[:], xhi_d[:, :, :, tile_pos[t]:C])
                for u in range(t, T):
                    x_sb[u] = xt
                    x_base[u] = tile_pos[u] - tile_pos[t]
                t = T
                break

        def emit_l1(t, h_dst, alt_pool=False):
            """Layer 1 for tile t -> h_hi (fp8, [128, 8, TT]) and
            h_lo (fp8, suffix columns only) in h_dst = (hhi, hlo)."""
            TT = tok_tiles[t]
            pos = tile_pos[t]
            hhi_t, hlo_t = h_dst
            xs = x_sb[t]
            xb = x_base[t]
            for j in range(8):
                pool, tag = ((pyp, "py") if alt_pool and j % 2 else
                             (php, "ph"))
                ph = pool.tile([128, 512], f32, tag=tag)
                # slab 0 hi first (full width, start=True)
                nc.tensor.matmul(ph[:, 0:TT], w1_sb[j][:, 0, 0],
                                 xs[:, 0, :, xb:xb + TT],
                                 start=True, stop=False, perf_mode=DR)
                for s in range(4):
                    if s > 0 and s < 3:
                        nc.tensor.matmul(ph[:, 0:TT], w1_sb[j][:, 0, s],
                                         xs[:, s, :, xb:xb + TT],
                                         start=False, stop=False,
                                         perf_mode=DR)
                    # W1 residual (global)
                    nc.tensor.matmul(ph[:, 0:TT], w1_sb[j][:, 1, s],
                                     xs[:, s, :, xb:xb + TT],
                                     start=False, stop=False, perf_mode=DR)
                    # x residual (suffix of highest-gate tokens)
                    u0 = max(0, (C - Kx[s]) - pos)
                    if u0 < TT and Kx[s] > 0:
                        i0 = pos + u0 - (C - Kxm)
                        nc.tensor.matmul(ph[:, u0:TT], w1_sb[j][:, 0, s],
                                         xlo_sb[:, s, :,
                                                i0:i0 + (TT - u0)],
                                         start=False, stop=False,
                                         perf_mode=DR)
                # slab 3 hi last (full width, stop=True)
                nc.tensor.matmul(ph[:, 0:TT], w1_sb[j][:, 0, 3],
                                 xs[:, 3, :, xb:xb + TT],
                                 start=False, stop=True, perf_mode=DR)
                # eviction: h8 = relu(psum*2^-5 + 8*b1) (= 8*h)
                hs = max(0, (C - Kh[j // 2]) - pos)
                if hs < TT:
                    # 3-op path: bf16 intermediate, fp8 copy, fp8 residual
                    hb = bpool.tile([128, TT], bf16, tag="hb")
                    nc.scalar.activation(hb[:], ph[:, 0:TT], RELU,
                                         bias=b1_sb[:, j:j + 1],
                                         scale=2.0 ** -5)
                    nc.scalar.activation(hhi_t[:, j, 0:TT], hb[:], COPY)
                    nc.vector.tensor_tensor(hlo_t[:, j, hs:TT],
                                            hb[:, hs:TT],
                                            hhi_t[:, j, hs:TT], op=SUB)
                else:
                    nc.scalar.activation(hhi_t[:, j, 0:TT], ph[:, 0:TT],
                                         RELU, bias=b1_sb[:, j:j + 1],
                                         scale=2.0 ** -5)

        def emit_l2(t, h_src, split_dma, tail=False):
            """Layer 2 for tile t from h_src = (hhi, hlo)."""
            TT = tok_tiles[t]
            pos = tile_pos[t]
            hhi_t, hlo_t = h_src
            yt = ypool.tile([128, 8 * TT], f16, tag="y")
            for o in range(8):
                pool = pyp if (not tail or o % 2 == 0) else php
                py = pool.tile([128, 512], f32,
                               tag=("py" if pool is pyp else "ph"))
                nc.tensor.matmul(py[:, 0:TT], w2_sb[o][:, 0, 0],
                                 hhi_t[:, 0:2, 0:TT],
                                 start=True, stop=False, perf_mode=DR)
                for s in range(4):
                    if s > 0 and s < 3:
                        nc.tensor.matmul(py[:, 0:TT], w2_sb[o][:, 0, s],
                                         hhi_t[:, 2 * s:2 * s + 2, 0:TT],
                                         start=False, stop=False,
                                         perf_mode=DR)
                    nc.tensor.matmul(py[:, 0:TT], w2_sb[o][:, 1, s],
                                     hhi_t[:, 2 * s:2 * s + 2, 0:TT],
                                     start=False, stop=False, perf_mode=DR)
                    u0 = max(0, (C - Kh[s]) - pos)
                    if u0 < TT and Kh[s] > 0:
                        nc.tensor.matmul(py[:, u0:TT], w2_sb[o][:, 0, s],
                                         hlo_t[:, 2 * s:2 * s + 2, u0:TT],
                                         start=False, stop=False,
                                         perf_mode=DR)
                nc.tensor.matmul(py[:, 0:TT], w2_sb[o][:, 0, 3],
                                 hhi_t[:, 6:8, 0:TT],
                                 start=False, stop=True, perf_mode=DR)
                # evict: y = (psum + 256*b2) * 2^-8 -> fp16
                nc.vector.tensor_scalar(yt[:, o * TT:(o + 1) * TT],
                                        py[:, 0:TT], b2_sb[:, o:o + 1],
                                        2.0 ** -8, op0=ADD, op1=MULT)
                if split_dma:
                    nc.sync.dma_start(yT_d[:, o, pos:pos + TT],
                                      yt[:, o * TT:(o + 1) * TT])
                if tail and o == 3:
                    nc.sync.dma_start(yE_d[:, 0:4 * TT], yt[:, 0:4 * TT])
            if not split_dma:
                if tail:
                    nc.sync.dma_start(yE_d[:, 4 * TT:], yt[:, 4 * TT:])
                else:
                    nc.sync.dma_start(yT_d[:, :, pos:pos + TT], yt[:])

        # PE section order: L1(0), L2(0), ..., with the (expensive) last
        # tile's L1 hoisted before L2(T-2) so its evictions hide under
        # matmuls, and tile T-2's output leaving per-o-strip.
        h_tiles = []
        for t in range(T):
            TT = tok_tiles[t]
            hhi_t = hpool.tile([128, 8, TT], f8, tag="hhi")
            hlo_t = hpool.tile([128, 8, TT], f8, tag="hlo")
            h_tiles.append((hhi_t, hlo_t))
        for t in range(T):
            if t < T - 1:
                emit_l1(t, h_tiles[t])
                if t == T - 2:
                    emit_l1(T - 1, h_tiles[T - 1], alt_pool=True)
                emit_l2(t, h_tiles[t], split_dma=(t == T - 2))
            else:
                if T == 1:
                    emit_l1(t, h_tiles[t])
                emit_l2(t, h_tiles[t], split_dma=False, tail=True)

    nc.compile()
    return nc


def _route(x, Wg, bg):
    """Host gating: fp32 softmax + top-2, matching jax.lax.top_k semantics."""
    logits = x @ Wg + bg
    m = logits.max(axis=1, keepdims=True)
    e = np.exp(logits - m)
    gates = e / e.sum(axis=1, keepdims=True)
    # stable argsort on negated values = ties broken by lower index (jax)
    order = np.argsort(-gates, axis=1, kind="stable")[:, :TOP_K]
    return gates, order


def _make_in_maps(x, W1, b1, W2, b2, gates, order, tok_lists, C, Kx, Kh,
                  TT0):
    import ml_dtypes
    f8 = ml_dtypes.float8_e4m3fn

    def q8(v):
        return np.ascontiguousarray(v).astype(f8)

    def deq(v):
        return v.astype(np.float32)

    def pack_w(W):
        # [1024, 1024] -> [8, 128, 2, 4, 2, 128] hi/lo strips
        Ws = W * 32.0
        # Wt[o/j, p, s, k, r] = Ws[(2s+k)*128+p, j*128+r]
        Wt = Ws.reshape(4, 2, 128, 8, 128).transpose(3, 2, 0, 1, 4)
        hi = Wt.astype(f8)
        lo = (Wt - deq(hi)).astype(f8)
        return np.ascontiguousarray(
            np.stack([hi, lo], axis=2))  # [8, 128, 2, 4, 2, 128]

    Kxm = max(max(Kx), 1)
    in_maps = []
    for e in range(NUM_EXPERTS):
        toks = tok_lists[e]
        ne = len(toks)
        # ascending gate sort, padding (zeros) in FRONT
        g = gates[toks, e]
        asc = toks[np.argsort(g, kind="stable")]
        xs = np.zeros((C, D), dtype=np.float32)
        xs[C - ne:] = x[asc]
        # xhi[p, s, k, c] = q(8*xs[c, (2s+k)*128+p])
        x8 = (xs * 8.0).reshape(C, 4, 2, 128)      # [c, s, k, p]
        xhi = x8.astype(f8)                        # quantize
        xhi_t = np.ascontiguousarray(xhi.transpose(3, 1, 2, 0))
        xres = x8 - deq(xhi)                       # [c, s, k, p]
        xlo = np.zeros((128, 4, 2, Kxm), dtype=f8)
        for s in range(4):
            k = Kx[s]
            if k:
                # [p, 2, c] from residual rows of the last k tokens
                blk = xres[C - k:, s].transpose(2, 1, 0)
                xlo[:, s, :, Kxm - k:] = q8(blk)
        bb = np.concatenate([8.0 * b1[e].reshape(8, 128).T,
                             256.0 * b2[e].reshape(8, 128).T], axis=1)
        in_maps.append({
            "w1": pack_w(W1[e]),
            "w2": pack_w(W2[e]),
            "x0": np.ascontiguousarray(xhi_t[:, :, :, 0:TT0]),
            "xhi": xhi_t,
            "xlo": xlo,
            "bb": np.ascontiguousarray(bb.astype(np.float32)),
        })
    return in_maps, [np.argsort(gates[tok_lists[e], e], kind="stable")
                     for e in range(NUM_EXPERTS)]


def kernel(x, W1, b1, W2, b2, Wg, bg):
    from concourse import bass_utils

    x = np.ascontiguousarray(np.asarray(x, dtype=np.float32))
    W1 = np.asarray(W1, dtype=np.float32)
    b1 = np.asarray(b1, dtype=np.float32)
    W2 = np.asarray(W2, dtype=np.float32)
    b2 = np.asarray(b2, dtype=np.float32)
    Wg = np.asarray(Wg, dtype=np.float32)
    bg = np.asarray(bg, dtype=np.float32)
    n = x.shape[0]

    gates, order = _route(x, Wg, bg)
    tok_lists = [np.where((order == e).any(axis=1))[0]
                 for e in range(NUM_EXPERTS)]
    max_load = max(len(t) for t in tok_lists)
    C, tok_tiles = _plan_tiles(max_load)
    Kx, Kh = _bounds(C)

    key = (C, tuple(tok_tiles), tuple(Kx), tuple(Kh))
    if key not in _prog_cache:
        _prog_cache[key] = _build_program(C, tok_tiles, Kx, Kh)
    nc = _prog_cache[key]

    in_maps, asc_orders = _make_in_maps(
        x, W1, b1, W2, b2, gates, order, tok_lists, C, Kx, Kh, tok_tiles[0])
    res = bass_utils.run_bass_kernel_spmd(nc, in_maps,
                                          list(range(NUM_EXPERTS)))

    TTe = tok_tiles[-1]
    out = np.zeros((n, D), dtype=np.float32)
    for e in range(NUM_EXPERTS):
        toks = tok_lists[e]
        ne = len(toks)
        yT = np.asarray(res.results[e]["yT"], dtype=np.float32)
        yE = np.asarray(res.results[e]["yE"], dtype=np.float32)
        yT[:, :, C - TTe:] = yE.reshape(128, 8, TTe)
        # yT[p, o, c] -> y[c, o*128+p]; positions C-ne.. hold the sorted toks
        y = yT[:, :, C - ne:].transpose(2, 1, 0).reshape(ne, D)
        asc = toks[asc_orders[e]]
        out[asc] += gates[asc, e][:, None] * y
    return out


# revision 18
# speedup vs baseline: 1.5198x; 1.0579x over previous
"""Trainium2 Bass kernel for an 8-expert top-2 MoE layer.

Strategy (expert-parallel + gate-adaptive fp8 precision ladder): the host
computes the (tiny) gating matmul + softmax + top-2 routing, gathers each
expert's assigned tokens SORTED BY GATE ASCENDING (padding in front), and
ships one expert per NeuronCore. All heavy compute runs as fp8e4m3
DoubleRow matmuls (0.5 cycles/row for K=256 vs bf16's 1.0 for K=128 —
4x PE throughput), with precision recovered via residual ("lo") fp8
correction terms:

  W ~= fp8(W*32) + fp8(W*32 - hi)     x ~= fp8(x*8) + fp8(x*8 - hi)

  L1 slab s (K=256): Whi@xhi + Wlo@xhi always; + Whi@xlo for the last
      Kx[s] (highest-gate) tokens.
  h eviction: h8 = relu(psum*2^-5 + 8*b1) -> fp8 directly (one ACT op);
      where h_lo is needed, a 3-op path (ACT->bf16, ACT copy->fp8,
      DVE subtract->fp8 residual).
  L2 slab s: W2hi@h_hi + W2lo@h_hi always; + W2hi@h_lo for the last
      Kh[s] tokens.
  y eviction: DVE (psum + 256*b2)*2^-8 -> fp16.

The per-(token,expert) quantization error is damped by that pair's gate
in the final combine, so low-gate tokens (the bulk; gates are flat-ish
~0.17-0.25) tolerate single-fp8 x/h operands while the few high-gate
tokens get full residual correction. Residual-term suffix boundaries
(Kx/Kh per slab, tuned in a host-side exact numerics simulator against
the 2e-2 absmax gate) make the ladder continuous: matmuls just cover
column sub-ranges of each tile, so no extra padding or segmenting.
Host-sim predicts 1.61e-2 absmax rel err (gate 2e-2); PE work is
~160k TT-cycles/core vs 232k for the previous bf16+partial-fp8 kernel.

All PSUM groups start and stop on full-width matmuls (suffix terms sit
in the middle) so partial-width accumulation is well-defined. Gate
multiply + top-2 combine stay on the host (exact fp32).
"""

import numpy as np

NUM_EXPERTS = 8
TOP_K = 2
D = 1024

# residual-term suffix boundaries as fractions of C (tuned in schedsim
# against the 2e-2 absmax gate): K*[s] = highest-gate token count getting
# that correction term for slab s. The W-side corrections cover almost
# everything (the lowest-gate ~15% of tokens tolerate raw W-noise); the
# x-lo/h-lo corrections only the high-gate head of the distribution.
FRAC_X = (0.2648, 0.1891, 0.1182, 0.0567)
FRAC_W = (0.8511, 0.8511, 0.8511, 0.8511)
FRAC_2 = (0.8511, 0.8511, 0.8511, 0.8511)
FRAC_H = (0.8511, 0.3783, 0.1182, 0.0567)

_prog_cache = {}


def _plan_tiles(max_load):
    """Token-tile sizes covering max_load exactly, ascending-gate order.

    Head tiles are 512 (one fp32 PSUM bank; wide enough that the L1
    strip groups don't outrun the w1 strip feed in the DMA-bound head).
    The remainder is split into two roughly equal tiles >= 128 at the
    (expensive, high-gate) tail so its L2/evictions overlap better.
    """
    C = max(int(max_load), 256)
    tiles = []
    rest = C
    while rest > 512:
        if rest - 512 >= 256 or rest - 512 == 0:
            tiles.append(512)
            rest -= 512
        else:
            a = rest // 2
            tiles.extend([rest - a, a])
            rest = 0
    if rest:
        tiles.append(rest)
    return C, tiles


def _bounds(C):
    Kx = [min(C, int(round(f * C))) for f in FRAC_X]
    Kw = [min(C, int(round(f * C))) for f in FRAC_W]
    K2 = [min(C, int(round(f * C))) for f in FRAC_2]
    Kh = [min(C, int(round(f * C))) for f in FRAC_H]
    return Kx, Kw, K2, Kh


def _build_program(C, tok_tiles, Kx, Kw, K2, Kh, n_warm=5, x0_split=False):
    """Per-core Bass program: one expert's 2-layer MLP over C tokens."""
    from contextlib import ExitStack

    import concourse.tile as tile
    from concourse import bacc, mybir

    f32 = mybir.dt.float32
    bf16 = mybir.dt.bfloat16
    f16 = mybir.dt.float16
    f8 = mybir.dt.float8e4
    DR = mybir.MatmulPerfMode.DoubleRow
    RELU = mybir.ActivationFunctionType.Relu
    COPY = mybir.ActivationFunctionType.Copy
    ADD = mybir.AluOpType.add
    MULT = mybir.AluOpType.mult
    SUB = mybir.AluOpType.subtract

    nc = bacc.Bacc("TRN2", target_bir_lowering=False, debug=False,
                   num_devices=NUM_EXPERTS)

    T = len(tok_tiles)
    tile_pos = [0]
    for TT in tok_tiles:
        tile_pos.append(tile_pos[-1] + TT)
    TT0 = tok_tiles[0]
    TTe = tok_tiles[-1]

    # host-packed layouts (ascending gate order, padding at the front):
    #   w1:  [8, 128, 2, 4, 2, 128]  w1[j, p, v, s, k, r] =
    #        q(32*W1[(2s+k)*128+p, j*128+r]) for v=0 (hi); residual v=1
    #   w2:  [8, 128, 2, 4, 2, 128]  same over W2 with j->o output strips
    #   x0:  [128, 4, 2, TT0]        head tile's xhi, own contiguous tensor
    #   xhi: [128, 4, 2, C]          xhi[p, s, k, c] = q(8*x_c[(2s+k)*128+p])
    #   xlo: [128, sum_s 2*Kx[s]]    per-slab suffix residuals, concatenated
    #   bb:  [128, 16]               [:, j] = 8*b1[j*128+p]; [:, 8+o] = 256*b2
    #   yT:  [128, 8, C] f16         yT[p, o, c] = y_c[o*128+p]
    #   yE:  [128, 8*TTe] f16        last tile's output (contiguous tail)
    w1_d = nc.dram_tensor("w1", [8, 128, 2, 4, 2, 128], f8,
                          kind="ExternalInput").ap()
    w2_d = nc.dram_tensor("w2", [8, 128, 2, 4, 2, 128], f8,
                          kind="ExternalInput").ap()
    x0_d = nc.dram_tensor("x0", [128, 4, 2, TT0], f8,
                          kind="ExternalInput").ap()
    xhi_d = nc.dram_tensor("xhi", [128, 4, 2, C], f8,
                           kind="ExternalInput").ap()
    # x residuals for the last Kxm token positions, all 4 slabs padded to
    # Kxm so slab slices are plain strided views (zeros where unused)
    Kxm = max(max(Kx), 1)
    xlo_d = nc.dram_tensor("xlo", [128, 4, 2, Kxm], f8,
                           kind="ExternalInput").ap()
    bb_d = nc.dram_tensor("bb", [128, 16], f32, kind="ExternalInput").ap()
    yT_d = nc.dram_tensor("yT", [128, 8, C], f16, kind="ExternalOutput").ap()
    yE_d = nc.dram_tensor("yE", [128, 8 * TTe], f16,
                          kind="ExternalOutput").ap()

    with tile.TileContext(nc) as tc, ExitStack() as ctx:
        wpool = ctx.enter_context(tc.tile_pool(name="w", bufs=1))
        cpool = ctx.enter_context(tc.tile_pool(name="const", bufs=1))
        xpool = ctx.enter_context(tc.tile_pool(name="x", bufs=1))
        hpool = ctx.enter_context(tc.tile_pool(name="h", bufs=2))
        bpool = ctx.enter_context(tc.tile_pool(name="hb", bufs=3))
        ypool = ctx.enter_context(tc.tile_pool(name="y", bufs=3))
        php = ctx.enter_context(tc.tile_pool(name="ph", bufs=4, space="PSUM"))
        pyp = ctx.enter_context(tc.tile_pool(name="py", bufs=4, space="PSUM"))

        # PE warm-up: on-chip zeros so dummy matmuls ride out the HAM clock
        # ramp while the DMA-bound head (w1 strip 0 + x0) lands.
        wz = cpool.tile([1, 640], bf16, tag="wz")
        nc.vector.memzero(wz[:])
        for _ in range(n_warm):
            warm = php.tile([128, 512], f32, tag="ph")
            nc.tensor.matmul(warm[:], wz[:, 0:128], wz[:, 128:640],
                             start=True, stop=True)

        # ---- DMA emission in consumption order ----
        # SP queue (strictly ordered): w1 strips, w2 strips, xlo, xhi
        # tiles 1..T-1, then per-tile y outputs as they are produced.
        # ACT queue: x0 (head tile) + biases, landing alongside w1 strip 0.
        w1_sb = []
        w1_0 = wpool.tile([128, 2, 4, 2, 128], f8, tag="w1_0")
        nc.sync.dma_start(w1_0[:], w1_d[0])
        w1_sb.append(w1_0)

        # x0 ships per-slab on the ACT queue so the first L1 group can
        # start as soon as w1 strip 0 + x0 slab 0 land (~2.5us), instead
        # of waiting for the whole 512KB tile.
        x0_sb = xpool.tile([128, 4, 2, TT0], f8, tag="x0")
        bb_sb = cpool.tile([128, 16], f32, tag="bb")
        if x0_split:
            nc.scalar.dma_start(x0_sb[:, 0], x0_d[:, 0])
            nc.scalar.dma_start(bb_sb[:], bb_d[:])
            nc.scalar.dma_start(x0_sb[:, 1:4], x0_d[:, 1:4])
        else:
            nc.scalar.dma_start(x0_sb[:], x0_d[:])
            nc.scalar.dma_start(bb_sb[:], bb_d[:])
        b1_sb = bb_sb[:, 0:8]
        b2_sb = bb_sb[:, 8:16]

        for j in range(1, 8):
            w1_j = wpool.tile([128, 2, 4, 2, 128], f8, tag=f"w1_{j}")
            nc.sync.dma_start(w1_j[:], w1_d[j])
            w1_sb.append(w1_j)
        w2_sb = []
        for o in range(8):
            w2_o = wpool.tile([128, 2, 4, 2, 128], f8, tag=f"w2_{o}")
            nc.sync.dma_start(w2_o[:], w2_d[o])
            w2_sb.append(w2_o)
        xlo_sb = xpool.tile([128, 4, 2, Kxm], f8, tag="xlo")
        nc.sync.dma_start(xlo_sb[:], xlo_d[:])

        # xhi tiles: tile 0 from x0; tiles 1.. from xhi_d slices, with
        # trailing sub-512 tiles grouped into one transfer (>=512B runs).
        x_sb = [None] * T
        x_base = [0] * T       # column offset of tile t inside its sb tile
        x_sb[0] = x0_sb
        t = 1
        while t < T:
            if tok_tiles[t] >= 512 or t == T - 1:
                xt = xpool.tile([128, 4, 2, tok_tiles[t]], f8, tag=f"x{t}")
                nc.sync.dma_start(
                    xt[:], xhi_d[:, :, :, tile_pos[t]:tile_pos[t + 1]])
                x_sb[t] = xt
                t += 1
            else:
                w = C - tile_pos[t]
                xt = xpool.tile([128, 4, 2, w], f8, tag=f"x{t}")
                nc.sync.dma_start(xt[:], xhi_d[:, :, :, tile_pos[t]:C])
                for u in range(t, T):
                    x_sb[u] = xt
                    x_base[u] = tile_pos[u] - tile_pos[t]
                t = T
                break

        def emit_l1(t, h_dst, alt_pool=False):
            """Layer 1 for tile t -> h_hi (fp8, [128, 8, TT]) and
            h_lo (fp8, suffix columns only) in h_dst = (hhi, hlo)."""
            TT = tok_tiles[t]
            pos = tile_pos[t]
            hhi_t, hlo_t = h_dst
            xs = x_sb[t]
            xb = x_base[t]
            for j in range(8):
                pool, tag = ((pyp, "py") if alt_pool and j % 2 else
                             (php, "ph"))
                ph = pool.tile([128, 512], f32, tag=tag)
                # slab 0 hi first (full width, start=True)
                nc.tensor.matmul(ph[:, 0:TT], w1_sb[j][:, 0, 0],
                                 xs[:, 0, :, xb:xb + TT],
                                 start=True, stop=False, perf_mode=DR)
                for s in range(4):
                    if s > 0 and s < 3:
                        nc.tensor.matmul(ph[:, 0:TT], w1_sb[j][:, 0, s],
                                         xs[:, s, :, xb:xb + TT],
                                         start=False, stop=False,
                                         perf_mode=DR)
                    # W1 residual (suffix)
                    v0 = max(0, (C - Kw[s]) - pos)
                    if v0 < TT and Kw[s] > 0:
                        nc.tensor.matmul(ph[:, v0:TT], w1_sb[j][:, 1, s],
                                         xs[:, s, :, xb + v0:xb + TT],
                                         start=False, stop=False,
                                         perf_mode=DR)
                    # x residual (suffix of highest-gate tokens)
                    u0 = max(0, (C - Kx[s]) - pos)
                    if u0 < TT and Kx[s] > 0:
                        i0 = pos + u0 - (C - Kxm)
                        nc.tensor.matmul(ph[:, u0:TT], w1_sb[j][:, 0, s],
                                         xlo_sb[:, s, :,
                                                i0:i0 + (TT - u0)],
                                         start=False, stop=False,
                                         perf_mode=DR)
                # slab 3 hi last (full width, stop=True)
                nc.tensor.matmul(ph[:, 0:TT], w1_sb[j][:, 0, 3],
                                 xs[:, 3, :, xb:xb + TT],
                                 start=False, stop=True, perf_mode=DR)
                # eviction: h8 = relu(psum*2^-5 + 8*b1) (= 8*h)
                hs = max(0, (C - Kh[j // 2]) - pos)
                if hs < TT:
                    # 3-op path: bf16 intermediate, fp8 copy, fp8 residual
                    hb = bpool.tile([128, TT], bf16, tag="hb")
                    nc.scalar.activation(hb[:], ph[:, 0:TT], RELU,
                                         bias=b1_sb[:, j:j + 1],
                                         scale=2.0 ** -5)
                    nc.scalar.activation(hhi_t[:, j, 0:TT], hb[:], COPY)
                    nc.vector.tensor_tensor(hlo_t[:, j, hs:TT],
                                            hb[:, hs:TT],
                                            hhi_t[:, j, hs:TT], op=SUB)
                else:
                    nc.scalar.activation(hhi_t[:, j, 0:TT], ph[:, 0:TT],
                                         RELU, bias=b1_sb[:, j:j + 1],
                                         scale=2.0 ** -5)

        def emit_l2(t, h_src, split_dma, tail=False):
            """Layer 2 for tile t from h_src = (hhi, hlo)."""
            TT = tok_tiles[t]
            pos = tile_pos[t]
            hhi_t, hlo_t = h_src
            yt = ypool.tile([128, 8 * TT], f16, tag="y")
            for o in range(8):
                pool = pyp if (not tail or o % 2 == 0) else php
                py = pool.tile([128, 512], f32,
                               tag=("py" if pool is pyp else "ph"))
                nc.tensor.matmul(py[:, 0:TT], w2_sb[o][:, 0, 0],
                                 hhi_t[:, 0:2, 0:TT],
                                 start=True, stop=False, perf_mode=DR)
                for s in range(4):
                    if s > 0 and s < 3:
                        nc.tensor.matmul(py[:, 0:TT], w2_sb[o][:, 0, s],
                                         hhi_t[:, 2 * s:2 * s + 2, 0:TT],
                                         start=False, stop=False,
                                         perf_mode=DR)
                    v0 = max(0, (C - K2[s]) - pos)
                    if v0 < TT and K2[s] > 0:
                        nc.tensor.matmul(py[:, v0:TT], w2_sb[o][:, 1, s],
                                         hhi_t[:, 2 * s:2 * s + 2, v0:TT],
                                         start=False, stop=False,
                                         perf_mode=DR)
                    u0 = max(0, (C - Kh[s]) - pos)
                    if u0 < TT and Kh[s] > 0:
                        nc.tensor.matmul(py[:, u0:TT], w2_sb[o][:, 0, s],
                                         hlo_t[:, 2 * s:2 * s + 2, u0:TT],
                                         start=False, stop=False,
                                         perf_mode=DR)
                nc.tensor.matmul(py[:, 0:TT], w2_sb[o][:, 0, 3],
                                 hhi_t[:, 6:8, 0:TT],
                                 start=False, stop=True, perf_mode=DR)
                # evict: y = (psum + 256*b2) * 2^-8 -> fp16
                nc.vector.tensor_scalar(yt[:, o * TT:(o + 1) * TT],
                                        py[:, 0:TT], b2_sb[:, o:o + 1],
                                        2.0 ** -8, op0=ADD, op1=MULT)
                if split_dma:
                    nc.sync.dma_start(yT_d[:, o, pos:pos + TT],
                                      yt[:, o * TT:(o + 1) * TT])
                if tail and o == 3:
                    nc.sync.dma_start(yE_d[:, 0:4 * TT], yt[:, 0:4 * TT])
                if tail and o == 6:
                    # leave only a tiny (1-strip) transfer on the critical
                    # tail after the final o=7 eviction
                    nc.sync.dma_start(yE_d[:, 4 * TT:7 * TT],
                                      yt[:, 4 * TT:7 * TT])
            if not split_dma:
                if tail:
                    nc.sync.dma_start(yE_d[:, 7 * TT:], yt[:, 7 * TT:])
                else:
                    nc.sync.dma_start(yT_d[:, :, pos:pos + TT], yt[:])

        # PE section order: L1(0), L2(0), ..., with the (expensive) last
        # tile's L1 hoisted before L2(T-2) so its evictions hide under
        # matmuls, and tile T-2's output leaving per-o-strip.
        h_tiles = []
        for t in range(T):
            TT = tok_tiles[t]
            hhi_t = hpool.tile([128, 8, TT], f8, tag="hhi")
            hlo_t = hpool.tile([128, 8, TT], f8, tag="hlo")
            h_tiles.append((hhi_t, hlo_t))
        for t in range(T):
            if t < T - 1:
                emit_l1(t, h_tiles[t])
                if t == T - 2:
                    emit_l1(T - 1, h_tiles[T - 1], alt_pool=True)
                emit_l2(t, h_tiles[t], split_dma=(t == T - 2))
            else:
                if T == 1:
                    emit_l1(t, h_tiles[t])
                emit_l2(t, h_tiles[t], split_dma=False, tail=True)

    nc.compile()
    return nc


def _route(x, Wg, bg):
    """Host gating: fp32 softmax + top-2, matching jax.lax.top_k semantics."""
    logits = x @ Wg + bg
    m = logits.max(axis=1, keepdims=True)
    e = np.exp(logits - m)
    gates = e / e.sum(axis=1, keepdims=True)
    # stable argsort on negated values = ties broken by lower index (jax)
    order = np.argsort(-gates, axis=1, kind="stable")[:, :TOP_K]
    return gates, order


def _make_in_maps(x, W1, b1, W2, b2, gates, order, tok_lists, C, Kx, Kh,
                  TT0):
    import ml_dtypes
    f8 = ml_dtypes.float8_e4m3fn

    def q8(v):
        return np.ascontiguousarray(v).astype(f8)

    def deq(v):
        return v.astype(np.float32)

    def pack_w(W):
        # [1024, 1024] -> [8, 128, 2, 4, 2, 128] hi/lo strips
        Ws = W * 32.0
        # Wt[o/j, p, s, k, r] = Ws[(2s+k)*128+p, j*128+r]
        Wt = Ws.reshape(4, 2, 128, 8, 128).transpose(3, 2, 0, 1, 4)
        hi = Wt.astype(f8)
        lo = (Wt - deq(hi)).astype(f8)
        return np.ascontiguousarray(
            np.stack([hi, lo], axis=2))  # [8, 128, 2, 4, 2, 128]

    Kxm = max(max(Kx), 1)
    in_maps = []
    for e in range(NUM_EXPERTS):
        toks = tok_lists[e]
        ne = len(toks)
        # ascending gate sort, padding (zeros) in FRONT
        g = gates[toks, e]
        asc = toks[np.argsort(g, kind="stable")]
        xs = np.zeros((C, D), dtype=np.float32)
        xs[C - ne:] = x[asc]
        # xhi[p, s, k, c] = q(8*xs[c, (2s+k)*128+p])
        x8 = (xs * 8.0).reshape(C, 4, 2, 128)      # [c, s, k, p]
        xhi = x8.astype(f8)                        # quantize
        xhi_t = np.ascontiguousarray(xhi.transpose(3, 1, 2, 0))
        xres = x8 - deq(xhi)                       # [c, s, k, p]
        xlo = np.zeros((128, 4, 2, Kxm), dtype=f8)
        for s in range(4):
            k = Kx[s]
            if k:
                # [p, 2, c] from residual rows of the last k tokens
                blk = xres[C - k:, s].transpose(2, 1, 0)
                xlo[:, s, :, Kxm - k:] = q8(blk)
        bb = np.concatenate([8.0 * b1[e].reshape(8, 128).T,
                             256.0 * b2[e].reshape(8, 128).T], axis=1)
        in_maps.append({
            "w1": pack_w(W1[e]),
            "w2": pack_w(W2[e]),
            "x0": np.ascontiguousarray(xhi_t[:, :, :, 0:TT0]),
            "xhi": xhi_t,
            "xlo": xlo,
            "bb": np.ascontiguousarray(bb.astype(np.float32)),
        })
    return in_maps, [np.argsort(gates[tok_lists[e], e], kind="stable")
                     for e in range(NUM_EXPERTS)]


def kernel(x, W1, b1, W2, b2, Wg, bg):
    from concourse import bass_utils

    x = np.ascontiguousarray(np.asarray(x, dtype=np.float32))
    W1 = np.asarray(W1, dtype=np.float32)
    b1 = np.asarray(b1, dtype=np.float32)
    W2 = np.asarray(W2, dtype=np.float32)
    b2 = np.asarray(b2, dtype=np.float32)
    Wg = np.asarray(Wg, dtype=np.float32)
    bg = np.asarray(bg, dtype=np.float32)
    n = x.shape[0]

    gates, order = _route(x, Wg, bg)
    tok_lists = [np.where((order == e).any(axis=1))[0]
                 for e in range(NUM_EXPERTS)]
    max_load = max(len(t) for t in tok_lists)
    C, tok_tiles = _plan_tiles(max_load)
    Kx, Kw, K2, Kh = _bounds(C)

    key = (C, tuple(tok_tiles), tuple(Kx), tuple(Kw), tuple(K2), tuple(Kh))
    if key not in _prog_cache:
        _prog_cache[key] = _build_program(C, tok_tiles, Kx, Kw, K2, Kh)
    nc = _prog_cache[key]

    in_maps, asc_orders = _make_in_maps(
        x, W1, b1, W2, b2, gates, order, tok_lists, C, Kx, Kh, tok_tiles[0])
    res = bass_utils.run_bass_kernel_spmd(nc, in_maps,
                                          list(range(NUM_EXPERTS)))

    TTe = tok_tiles[-1]
    out = np.zeros((n, D), dtype=np.float32)
    for e in range(NUM_EXPERTS):
        toks = tok_lists[e]
        ne = len(toks)
        yT = np.asarray(res.results[e]["yT"], dtype=np.float32)
        yE = np.asarray(res.results[e]["yE"], dtype=np.float32)
        yT[:, :, C - TTe:] = yE.reshape(128, 8, TTe)
        # yT[p, o, c] -> y[c, o*128+p]; positions C-ne.. hold the sorted toks
        y = yT[:, :, C - ne:].transpose(2, 1, 0).reshape(ne, D)
        asc = toks[asc_orders[e]]
        out[asc] += gates[asc, e][:, None] * y
    return out


# revision 19
# speedup vs baseline: 1.5200x; 1.0001x over previous
"""Trainium2 Bass kernel for an 8-expert top-2 MoE layer.

Strategy (expert-parallel + gate-adaptive fp8 precision ladder): the host
computes the (tiny) gating matmul + softmax + top-2 routing, gathers each
expert's assigned tokens SORTED BY GATE ASCENDING (padding in front), and
ships one expert per NeuronCore. All heavy compute runs as fp8e4m3
DoubleRow matmuls (0.5 cycles/row for K=256 vs bf16's 1.0 for K=128 —
4x PE throughput), with precision recovered via residual ("lo") fp8
correction terms:

  W ~= fp8(W*32) + fp8(W*32 - hi)     x ~= fp8(x*8) + fp8(x*8 - hi)

  L1 slab s (K=256): Whi@xhi + Wlo@xhi always; + Whi@xlo for the last
      Kx[s] (highest-gate) tokens.
  h eviction: h8 = relu(psum*2^-5 + 8*b1) -> fp8 directly (one ACT op);
      where h_lo is needed, a 3-op path (ACT->bf16, ACT copy->fp8,
      DVE subtract->fp8 residual).
  L2 slab s: W2hi@h_hi + W2lo@h_hi always; + W2hi@h_lo for the last
      Kh[s] tokens.
  y eviction: DVE (psum + 256*b2)*2^-8 -> fp16.

The per-(token,expert) quantization error is damped by that pair's gate
in the final combine, so low-gate tokens (the bulk; gates are flat-ish
~0.17-0.25) tolerate single-fp8 x/h operands while the few high-gate
tokens get full residual correction. Residual-term suffix boundaries
(Kx/Kh per slab, tuned in a host-side exact numerics simulator against
the 2e-2 absmax gate) make the ladder continuous: matmuls just cover
column sub-ranges of each tile, so no extra padding or segmenting.
Host-sim predicts 1.61e-2 absmax rel err (gate 2e-2); PE work is
~160k TT-cycles/core vs 232k for the previous bf16+partial-fp8 kernel.

All PSUM groups start and stop on full-width matmuls (suffix terms sit
in the middle) so partial-width accumulation is well-defined. Gate
multiply + top-2 combine stay on the host (exact fp32).
"""

import numpy as np

NUM_EXPERTS = 8
TOP_K = 2
D = 1024

# residual-term suffix boundaries as fractions of C (tuned in schedsim
# against the 2e-2 absmax gate): K*[s] = highest-gate token count getting
# that correction term for slab s. The W-side corrections cover almost
# everything (the lowest-gate ~15% of tokens tolerate raw W-noise); the
# x-lo/h-lo corrections only the high-gate head of the distribution.
FRAC_X = (0.2648, 0.1891, 0.1182, 0.0567)
FRAC_W = (0.8511, 0.8511, 0.8511, 0.8511)
FRAC_2 = (0.8511, 0.8511, 0.8511, 0.8511)
FRAC_H = (0.8511, 0.3783, 0.1182, 0.0567)

_prog_cache = {}


def _plan_tiles(max_load):
    """Token-tile sizes covering max_load exactly, ascending-gate order.

    Head tiles are 512 (one fp32 PSUM bank; wide enough that the L1
    strip groups don't outrun the w1 strip feed in the DMA-bound head).
    The remainder is split into two roughly equal tiles >= 128 at the
    (expensive, high-gate) tail so its L2/evictions overlap better.
    """
    C = max(int(max_load), 256)
    tiles = []
    rest = C
    while rest > 512:
        if rest - 512 >= 256 or rest - 512 == 0:
            tiles.append(512)
            rest -= 512
        else:
            a = rest // 2
            tiles.extend([rest - a, a])
            rest = 0
    if rest:
        tiles.append(rest)
    return C, tiles


def _bounds(C):
    Kx = [min(C, int(round(f * C))) for f in FRAC_X]
    Kw = [min(C, int(round(f * C))) for f in FRAC_W]
    K2 = [min(C, int(round(f * C))) for f in FRAC_2]
    Kh = [min(C, int(round(f * C))) for f in FRAC_H]
    return Kx, Kw, K2, Kh


def _build_program(C, tok_tiles, Kx, Kw, K2, Kh, n_warm=5, x0_split=False):
    """Per-core Bass program: one expert's 2-layer MLP over C tokens."""
    from contextlib import ExitStack

    import concourse.tile as tile
    from concourse import bacc, mybir

    f32 = mybir.dt.float32
    bf16 = mybir.dt.bfloat16
    f16 = mybir.dt.float16
    f8 = mybir.dt.float8e4
    DR = mybir.MatmulPerfMode.DoubleRow
    RELU = mybir.ActivationFunctionType.Relu
    COPY = mybir.ActivationFunctionType.Copy
    ADD = mybir.AluOpType.add
    MULT = mybir.AluOpType.mult
    SUB = mybir.AluOpType.subtract

    nc = bacc.Bacc("TRN2", target_bir_lowering=False, debug=False,
                   num_devices=NUM_EXPERTS)

    T = len(tok_tiles)
    tile_pos = [0]
    for TT in tok_tiles:
        tile_pos.append(tile_pos[-1] + TT)
    TT0 = tok_tiles[0]
    TTe = tok_tiles[-1]

    # host-packed layouts (ascending gate order, padding at the front):
    #   w1:  [8, 128, 2, 4, 2, 128]  w1[j, p, v, s, k, r] =
    #        q(32*W1[(2s+k)*128+p, j*128+r]) for v=0 (hi); residual v=1
    #   w2:  [8, 128, 2, 4, 2, 128]  same over W2 with j->o output strips
    #   x0:  [128, 4, 2, TT0]        head tile's xhi, own contiguous tensor
    #   xhi: [128, 4, 2, C]          xhi[p, s, k, c] = q(8*x_c[(2s+k)*128+p])
    #   xlo: [128, sum_s 2*Kx[s]]    per-slab suffix residuals, concatenated
    #   bb:  [128, 16]               [:, j] = 8*b1[j*128+p]; [:, 8+o] = 256*b2
    #   yT:  [128, 8, C] f16         yT[p, o, c] = y_c[o*128+p]
    #   yE:  [128, 8*TTe] f16        last tile's output (contiguous tail)
    w1_d = nc.dram_tensor("w1", [8, 128, 2, 4, 2, 128], f8,
                          kind="ExternalInput").ap()
    w2_d = nc.dram_tensor("w2", [8, 128, 2, 4, 2, 128], f8,
                          kind="ExternalInput").ap()
    x0_d = nc.dram_tensor("x0", [128, 4, 2, TT0], f8,
                          kind="ExternalInput").ap()
    xhi_d = nc.dram_tensor("xhi", [128, 4, 2, C], f8,
                           kind="ExternalInput").ap()
    # x residuals for the last Kxm token positions, all 4 slabs padded to
    # Kxm so slab slices are plain strided views (zeros where unused)
    Kxm = max(max(Kx), 1)
    xlo_d = nc.dram_tensor("xlo", [128, 4, 2, Kxm], f8,
                           kind="ExternalInput").ap()
    bb_d = nc.dram_tensor("bb", [128, 16], f32, kind="ExternalInput").ap()
    yT_d = nc.dram_tensor("yT", [128, 8, C], f16, kind="ExternalOutput").ap()
    yE_d = nc.dram_tensor("yE", [128, 8 * TTe], f16,
                          kind="ExternalOutput").ap()

    with tile.TileContext(nc) as tc, ExitStack() as ctx:
        wpool = ctx.enter_context(tc.tile_pool(name="w", bufs=1))
        cpool = ctx.enter_context(tc.tile_pool(name="const", bufs=1))
        xpool = ctx.enter_context(tc.tile_pool(name="x", bufs=1))
        hpool = ctx.enter_context(tc.tile_pool(name="h", bufs=2))
        bpool = ctx.enter_context(tc.tile_pool(name="hb", bufs=3))
        ypool = ctx.enter_context(tc.tile_pool(name="y", bufs=3))
        php = ctx.enter_context(tc.tile_pool(name="ph", bufs=4, space="PSUM"))
        pyp = ctx.enter_context(tc.tile_pool(name="py", bufs=4, space="PSUM"))

        # PE warm-up: on-chip zeros so dummy matmuls ride out the HAM clock
        # ramp while the DMA-bound head (w1 strip 0 + x0) lands.
        wz = cpool.tile([1, 640], bf16, tag="wz")
        nc.vector.memzero(wz[:])
        for _ in range(n_warm):
            warm = php.tile([128, 512], f32, tag="ph")
            nc.tensor.matmul(warm[:], wz[:, 0:128], wz[:, 128:640],
                             start=True, stop=True)

        # ---- DMA emission in consumption order ----
        # SP queue (strictly ordered): w1 strips, w2 strips, xlo, xhi
        # tiles 1..T-1, then per-tile y outputs as they are produced.
        # ACT queue: x0 (head tile) + biases, landing alongside w1 strip 0.
        w1_sb = []
        w1_0 = wpool.tile([128, 2, 4, 2, 128], f8, tag="w1_0")
        nc.sync.dma_start(w1_0[:], w1_d[0])
        w1_sb.append(w1_0)

        # x0 ships per-slab on the ACT queue so the first L1 group can
        # start as soon as w1 strip 0 + x0 slab 0 land (~2.5us), instead
        # of waiting for the whole 512KB tile.
        x0_sb = xpool.tile([128, 4, 2, TT0], f8, tag="x0")
        bb_sb = cpool.tile([128, 16], f32, tag="bb")
        if x0_split:
            nc.scalar.dma_start(x0_sb[:, 0], x0_d[:, 0])
            nc.scalar.dma_start(bb_sb[:], bb_d[:])
            nc.scalar.dma_start(x0_sb[:, 1:4], x0_d[:, 1:4])
        else:
            nc.scalar.dma_start(x0_sb[:], x0_d[:])
            nc.scalar.dma_start(bb_sb[:], bb_d[:])
        b1_sb = bb_sb[:, 0:8]
        b2_sb = bb_sb[:, 8:16]

        for j in range(1, 8):
            w1_j = wpool.tile([128, 2, 4, 2, 128], f8, tag=f"w1_{j}")
            nc.sync.dma_start(w1_j[:], w1_d[j])
            w1_sb.append(w1_j)
        w2_sb = []
        for o in range(8):
            w2_o = wpool.tile([128, 2, 4, 2, 128], f8, tag=f"w2_{o}")
            nc.sync.dma_start(w2_o[:], w2_d[o])
            w2_sb.append(w2_o)
        xlo_sb = xpool.tile([128, 4, 2, Kxm], f8, tag="xlo")
        nc.sync.dma_start(xlo_sb[:], xlo_d[:])

        # xhi tiles: tile 0 from x0; tiles 1.. from xhi_d slices, with
        # trailing sub-512 tiles grouped into one transfer (>=512B runs).
        x_sb = [None] * T
        x_base = [0] * T       # column offset of tile t inside its sb tile
        x_sb[0] = x0_sb
        t = 1
        while t < T:
            if tok_tiles[t] >= 512 or t == T - 1:
                xt = xpool.tile([128, 4, 2, tok_tiles[t]], f8, tag=f"x{t}")
                nc.sync.dma_start(
                    xt[:], xhi_d[:, :, :, tile_pos[t]:tile_pos[t + 1]])
                x_sb[t] = xt
                t += 1
            else:
                w = C - tile_pos[t]
                xt = xpool.tile([128, 4, 2, w], f8, tag=f"x{t}")
                nc.sync.dma_start(xt[:], xhi_d[:, :, :, tile_pos[t]:C])
                for u in range(t, T):
                    x_sb[u] = xt
                    x_base[u] = tile_pos[u] - tile_pos[t]
                t = T
                break

        def emit_l1(t, h_dst, alt_pool=False):
            """Layer 1 for tile t -> h_hi (fp8, [128, 8, TT]) and
            h_lo (fp8, suffix columns only) in h_dst = (hhi, hlo)."""
            TT = tok_tiles[t]
            pos = tile_pos[t]
            hhi_t, hlo_t = h_dst
            xs = x_sb[t]
            xb = x_base[t]
            for j in range(8):
                pool, tag = ((pyp, "py") if alt_pool and j % 2 else
                             (php, "ph"))
                ph = pool.tile([128, 512], f32, tag=tag)
                # slab 0 hi first (full width, start=True)
                nc.tensor.matmul(ph[:, 0:TT], w1_sb[j][:, 0, 0],
                                 xs[:, 0, :, xb:xb + TT],
                                 start=True, stop=False, perf_mode=DR)
                for s in range(4):
                    if s > 0 and s < 3:
                        nc.tensor.matmul(ph[:, 0:TT], w1_sb[j][:, 0, s],
                                         xs[:, s, :, xb:xb + TT],
                                         start=False, stop=False,
                                         perf_mode=DR)
                    # W1 residual (suffix)
                    v0 = max(0, (C - Kw[s]) - pos)
                    if v0 < TT and Kw[s] > 0:
                        nc.tensor.matmul(ph[:, v0:TT], w1_sb[j][:, 1, s],
                                         xs[:, s, :, xb + v0:xb + TT],
                                         start=False, stop=False,
                                         perf_mode=DR)
                    # x residual (suffix of highest-gate tokens)
                    u0 = max(0, (C - Kx[s]) - pos)
                    if u0 < TT and Kx[s] > 0:
                        i0 = pos + u0 - (C - Kxm)
                        nc.tensor.matmul(ph[:, u0:TT], w1_sb[j][:, 0, s],
                                         xlo_sb[:, s, :,
                                                i0:i0 + (TT - u0)],
                                         start=False, stop=False,
                                         perf_mode=DR)
                # slab 3 hi last (full width, stop=True)
                nc.tensor.matmul(ph[:, 0:TT], w1_sb[j][:, 0, 3],
                                 xs[:, 3, :, xb:xb + TT],
                                 start=False, stop=True, perf_mode=DR)
                # eviction: h8 = relu(psum*2^-5 + 8*b1) (= 8*h). Tokens in
                # the h-lo suffix go through a bf16 intermediate (so the
                # fp8 residual can be formed); tokens before it evict
                # straight to fp8 — exactly matching the host simulator.
                hs = max(0, (C - Kh[j // 2]) - pos)
                if hs < TT:
                    if hs > 0:
                        nc.scalar.activation(hhi_t[:, j, 0:hs],
                                             ph[:, 0:hs], RELU,
                                             bias=b1_sb[:, j:j + 1],
                                             scale=2.0 ** -5)
                    hb = bpool.tile([128, TT], bf16, tag="hb")
                    nc.scalar.activation(hb[:, 0:TT - hs], ph[:, hs:TT],
                                         RELU, bias=b1_sb[:, j:j + 1],
                                         scale=2.0 ** -5)
                    nc.scalar.activation(hhi_t[:, j, hs:TT],
                                         hb[:, 0:TT - hs], COPY)
                    nc.vector.tensor_tensor(hlo_t[:, j, hs:TT],
                                            hb[:, 0:TT - hs],
                                            hhi_t[:, j, hs:TT], op=SUB)
                else:
                    nc.scalar.activation(hhi_t[:, j, 0:TT], ph[:, 0:TT],
                                         RELU, bias=b1_sb[:, j:j + 1],
                                         scale=2.0 ** -5)

        def emit_l2(t, h_src, split_dma, tail=False):
            """Layer 2 for tile t from h_src = (hhi, hlo)."""
            TT = tok_tiles[t]
            pos = tile_pos[t]
            hhi_t, hlo_t = h_src
            yt = ypool.tile([128, 8 * TT], f16, tag="y")
            for o in range(8):
                pool = pyp if (not tail or o % 2 == 0) else php
                py = pool.tile([128, 512], f32,
                               tag=("py" if pool is pyp else "ph"))
                nc.tensor.matmul(py[:, 0:TT], w2_sb[o][:, 0, 0],
                                 hhi_t[:, 0:2, 0:TT],
                                 start=True, stop=False, perf_mode=DR)
                for s in range(4):
                    if s > 0 and s < 3:
                        nc.tensor.matmul(py[:, 0:TT], w2_sb[o][:, 0, s],
                                         hhi_t[:, 2 * s:2 * s + 2, 0:TT],
                                         start=False, stop=False,
                                         perf_mode=DR)
                    v0 = max(0, (C - K2[s]) - pos)
                    if v0 < TT and K2[s] > 0:
                        nc.tensor.matmul(py[:, v0:TT], w2_sb[o][:, 1, s],
                                         hhi_t[:, 2 * s:2 * s + 2, v0:TT],
                                         start=False, stop=False,
                                         perf_mode=DR)
                    u0 = max(0, (C - Kh[s]) - pos)
                    if u0 < TT and Kh[s] > 0:
                        nc.tensor.matmul(py[:, u0:TT], w2_sb[o][:, 0, s],
                                         hlo_t[:, 2 * s:2 * s + 2, u0:TT],
                                         start=False, stop=False,
                                         perf_mode=DR)
                nc.tensor.matmul(py[:, 0:TT], w2_sb[o][:, 0, 3],
                                 hhi_t[:, 6:8, 0:TT],
                                 start=False, stop=True, perf_mode=DR)
                # evict: y = (psum + 256*b2) * 2^-8 -> fp16
                nc.vector.tensor_scalar(yt[:, o * TT:(o + 1) * TT],
                                        py[:, 0:TT], b2_sb[:, o:o + 1],
                                        2.0 ** -8, op0=ADD, op1=MULT)
                if split_dma:
                    nc.sync.dma_start(yT_d[:, o, pos:pos + TT],
                                      yt[:, o * TT:(o + 1) * TT])
                if tail and o == 3:
                    nc.sync.dma_start(yE_d[:, 0:4 * TT], yt[:, 0:4 * TT])
                if tail and o == 6:
                    # leave only a tiny (1-strip) transfer on the critical
                    # tail after the final o=7 eviction
                    nc.sync.dma_start(yE_d[:, 4 * TT:7 * TT],
                                      yt[:, 4 * TT:7 * TT])
            if not split_dma:
                if tail:
                    nc.sync.dma_start(yE_d[:, 7 * TT:], yt[:, 7 * TT:])
                else:
                    nc.sync.dma_start(yT_d[:, :, pos:pos + TT], yt[:])

        # PE section order: L1(0), L2(0), ..., with the (expensive) last
        # tile's L1 hoisted before L2(T-2) so its evictions hide under
        # matmuls, and tile T-2's output leaving per-o-strip.
        h_tiles = []
        for t in range(T):
            TT = tok_tiles[t]
            hhi_t = hpool.tile([128, 8, TT], f8, tag="hhi")
            hlo_t = hpool.tile([128, 8, TT], f8, tag="hlo")
            h_tiles.append((hhi_t, hlo_t))
        for t in range(T):
            if t < T - 1:
                emit_l1(t, h_tiles[t])
                if t == T - 2:
                    emit_l1(T - 1, h_tiles[T - 1], alt_pool=True)
                emit_l2(t, h_tiles[t], split_dma=(t == T - 2))
            else:
                if T == 1:
                    emit_l1(t, h_tiles[t])
                emit_l2(t, h_tiles[t], split_dma=False, tail=True)

    nc.compile()
    return nc


def _route(x, Wg, bg):
    """Host gating: fp32 softmax + top-2, matching jax.lax.top_k semantics."""
    logits = x @ Wg + bg
    m = logits.max(axis=1, keepdims=True)
    e = np.exp(logits - m)
    gates = e / e.sum(axis=1, keepdims=True)
    # stable argsort on negated values = ties broken by lower index (jax)
    order = np.argsort(-gates, axis=1, kind="stable")[:, :TOP_K]
    return gates, order


def _make_in_maps(x, W1, b1, W2, b2, gates, order, tok_lists, C, Kx, Kh,
                  TT0):
    import ml_dtypes
    f8 = ml_dtypes.float8_e4m3fn

    def q8(v):
        return np.ascontiguousarray(v).astype(f8)

    def deq(v):
        return v.astype(np.float32)

    def pack_w(W):
        # [1024, 1024] -> [8, 128, 2, 4, 2, 128] hi/lo strips
        Ws = W * 32.0
        # Wt[o/j, p, s, k, r] = Ws[(2s+k)*128+p, j*128+r]
        Wt = Ws.reshape(4, 2, 128, 8, 128).transpose(3, 2, 0, 1, 4)
        hi = Wt.astype(f8)
        lo = (Wt - deq(hi)).astype(f8)
        return np.ascontiguousarray(
            np.stack([hi, lo], axis=2))  # [8, 128, 2, 4, 2, 128]

    Kxm = max(max(Kx), 1)
    in_maps = []
    for e in range(NUM_EXPERTS):
        toks = tok_lists[e]
        ne = len(toks)
        # ascending gate sort, padding (zeros) in FRONT
        g = gates[toks, e]
        asc = toks[np.argsort(g, kind="stable")]
        xs = np.zeros((C, D), dtype=np.float32)
        xs[C - ne:] = x[asc]
        # xhi[p, s, k, c] = q(8*xs[c, (2s+k)*128+p])
        x8 = (xs * 8.0).reshape(C, 4, 2, 128)      # [c, s, k, p]
        xhi = x8.astype(f8)                        # quantize
        xhi_t = np.ascontiguousarray(xhi.transpose(3, 1, 2, 0))
        xres = x8 - deq(xhi)                       # [c, s, k, p]
        xlo = np.zeros((128, 4, 2, Kxm), dtype=f8)
        for s in range(4):
            k = Kx[s]
            if k:
                # [p, 2, c] from residual rows of the last k tokens
                blk = xres[C - k:, s].transpose(2, 1, 0)
                xlo[:, s, :, Kxm - k:] = q8(blk)
        bb = np.concatenate([8.0 * b1[e].reshape(8, 128).T,
                             256.0 * b2[e].reshape(8, 128).T], axis=1)
        in_maps.append({
            "w1": pack_w(W1[e]),
            "w2": pack_w(W2[e]),
            "x0": np.ascontiguousarray(xhi_t[:, :, :, 0:TT0]),
            "xhi": xhi_t,
            "xlo": xlo,
            "bb": np.ascontiguousarray(bb.astype(np.float32)),
        })
    return in_maps, [np.argsort(gates[tok_lists[e], e], kind="stable")
                     for e in range(NUM_EXPERTS)]


def kernel(x, W1, b1, W2, b2, Wg, bg):
    from concourse import bass_utils

    x = np.ascontiguousarray(np.asarray(x, dtype=np.float32))
    W1 = np.asarray(W1, dtype=np.float32)
    b1 = np.asarray(b1, dtype=np.float32)
    W2 = np.asarray(W2, dtype=np.float32)
    b2 = np.asarray(b2, dtype=np.float32)
    Wg = np.asarray(Wg, dtype=np.float32)
    bg = np.asarray(bg, dtype=np.float32)
    n = x.shape[0]

    gates, order = _route(x, Wg, bg)
    tok_lists = [np.where((order == e).any(axis=1))[0]
                 for e in range(NUM_EXPERTS)]
    max_load = max(len(t) for t in tok_lists)
    C, tok_tiles = _plan_tiles(max_load)
    Kx, Kw, K2, Kh = _bounds(C)

    key = (C, tuple(tok_tiles), tuple(Kx), tuple(Kw), tuple(K2), tuple(Kh))
    if key not in _prog_cache:
        _prog_cache[key] = _build_program(C, tok_tiles, Kx, Kw, K2, Kh)
    nc = _prog_cache[key]

    in_maps, asc_orders = _make_in_maps(
        x, W1, b1, W2, b2, gates, order, tok_lists, C, Kx, Kh, tok_tiles[0])
    res = bass_utils.run_bass_kernel_spmd(nc, in_maps,
                                          list(range(NUM_EXPERTS)))

    TTe = tok_tiles[-1]
    out = np.zeros((n, D), dtype=np.float32)
    for e in range(NUM_EXPERTS):
        toks = tok_lists[e]
        ne = len(toks)
        yT = np.asarray(res.results[e]["yT"], dtype=np.float32)
        yE = np.asarray(res.results[e]["yE"], dtype=np.float32)
        yT[:, :, C - TTe:] = yE.reshape(128, 8, TTe)
        # yT[p, o, c] -> y[c, o*128+p]; positions C-ne.. hold the sorted toks
        y = yT[:, :, C - ne:].transpose(2, 1, 0).reshape(ne, D)
        asc = toks[asc_orders[e]]
        out[asc] += gates[asc, e][:, None] * y
    return out
